# revision 24
# baseline (speedup 1.0000x reference)
"""DeepJ (TimeAxis + NoteAxis LSTM) Trainium2 kernel, v2.

Data-parallel over 8 NeuronCores: batch 1024 -> 128 per core.

Layout: activations are [units, rows] tiles with rows = (note, batch) on the
free dimension; weights are the stationary (lhsT) matmul operands, so the
NoteAxis recurrence needs no per-step transposes.

v2 structure (vs v1):
  * NoteAxis input projections (Wih0 @ [nf; shifted] + bias) are batched into
    the feed-forward phase as N=512 matmuls and stored in SBUF (pre0, bf16).
    Each NA step loads them into PSUM with ONE identity-matmul that also
    opens the bank's single accumulation group -- the v1 bug was four
    interleaved per-chunk groups in one bank: start=True clears the whole
    bank's has_written bits, so later start=False closers overwrote the
    openers' data instead of accumulating.
  * The K=4 shifted-notes projection and the chord broadcast are computed on
    the host (numpy); output bias + sigmoid applied on host as well.
  * Output projection is 12 [M=3, N=512] matmuls instead of 48 [*, N=3].
  * h0/nf TA tiles are per-block rings instead of full-R persists.

Matmul dtype is bfloat16 throughout (f32r lowers to 2x hi/lo passes
on this target, doubling PE time for no useful accuracy here).
"""

import os
import sys

for _p in ("/opt/trn_rl_repo",):
    if _p not in sys.path:
        sys.path.insert(0, _p)

import numpy as np

# ---- model constants -------------------------------------------------------
N_CORES = 8
B_TOT = 1024
B = B_TOT // N_CORES          # 128 rows per core
NN = 48                       # notes
OCT = 12
R = NN * B                    # 6144 rows, ordered (note, batch)
NBLK = 12                     # row blocks of 512 for the feed-forward stages
BLK = 512

_PROGRAM_CACHE = {}
_DBG = bool(os.environ.get("DEEPJ_DEBUG"))


def _build_program():
    import concourse.tile as tile
    from concourse import bacc, mybir

    f32 = mybir.dt.float32
    f32r = mybir.dt.float32r
    bf16 = mybir.dt.bfloat16

    nc = bacc.Bacc(
        "TRN2", target_bir_lowering=False, debug=False, num_devices=N_CORES
    )

    def param(name, shape, dtype=f32):
        return nc.declare_dram_parameter(name, list(shape), dtype, isOutput=False)

    P = {}
    # per-core activations / gathered inputs
    P["im2colT"] = param("im2colT", [75, R], bf16)   # conv patches, (c*25+s, (n,b))
    P["beat_bc"] = param("beat_bc", [16, R], bf16)   # beat_in^T bcast over n
    P["e48"] = param("e48", [48, R], bf16)           # one-hot(n) bcast over b
    P["chord_bc"] = param("chord_bc", [12, R], bf16)  # chord^T bcast over n
    P["presh"] = param("presh", [128, NN * 512], bf16)  # Wsh@shifted+b0, (n,q,b)
    # weights (replicated on every core)
    P["w0comb"] = param("w0comb", [108, 768], bf16)  # folded TA-L0 lhsT
    P["lvic"] = param("lvic", [75, 32], bf16)        # conv lhsT
    P["vicb"] = param("vicb", [32, 1])
    P["w1dr"] = param("w1dr", [128, 2 * 768], mybir.dt.float8e4)  # TA-L1 DoubleRow lhsT
    P["b1t"] = param("b1t", [128, 6])                # TA-L1 bias per u-chunk
    P["lnfdr"] = param("lnfdr", [128, 2 * 512], mybir.dt.float8e4)  # NA-L0 Wih DoubleRow lhsT
    P["lhh0"] = param("lhh0", [128, 512], bf16)      # NA-L0 Whh lhsT
    P["lih1"] = param("lih1", [128, 512], bf16)      # NA-L1 Wih lhsT
    P["lhh1"] = param("lhh1", [128, 512], bf16)      # NA-L1 Whh lhsT
    P["ident"] = param("ident", [128, 128], bf16)    # identity (psum preload)
    P["nb1bc"] = param("nb1bc", [128, 512], bf16)    # NA-L1 bias, (q,b) bcast
    P["outWT"] = param("outWT", [128, 3], bf16)
    P["y3"] = nc.declare_dram_parameter("y3", [3, R], bf16, isOutput=True)
    if _DBG:
        for nm, shp, dt in [("d_h1", [128, R], bf16),
                            ("d_g0", [128, 512 * NN], f32),
                            ("d_s0", [128, 512 * NN], f32),
                            ("d_h0na", [128, B * NN], bf16),
                            ("d_pre0", [128, NN * 512], bf16)]:
            P[nm] = nc.declare_dram_parameter(nm, shp, dt, isOutput=True)

    with tile.TileContext(nc) as tc:
        _emit(nc, tc, mybir, P)
    nc.compile()
    return nc


def _emit(nc, tc, mybir, P):
    from contextlib import ExitStack

    f32 = mybir.dt.float32
    f32r = mybir.dt.float32r
    bf16 = mybir.dt.bfloat16
    AF = mybir.ActivationFunctionType
    Alu = mybir.AluOpType

    with ExitStack() as top:
        wpool = top.enter_context(tc.tile_pool(name="weights", bufs=1))
        persist = top.enter_context(tc.tile_pool(name="persist", bufs=1))
        h0pool = top.enter_context(tc.tile_pool(name="h0ring2", bufs=2))
        nfpool = top.enter_context(tc.tile_pool(name="nfring", bufs=2))
        scr = top.enter_context(tc.tile_pool(name="scr", bufs=2))
        nascr = top.enter_context(tc.tile_pool(name="nascr", bufs=3))
        h0ring = top.enter_context(tc.tile_pool(name="h0ring", bufs=3))
        cpool = top.enter_context(tc.tile_pool(name="cstate", bufs=2))
        im_pool = top.enter_context(tc.tile_pool(name="im", bufs=3))
        pta = top.enter_context(tc.tile_pool(name="pta", bufs=1, space="PSUM"))
        ppp = top.enter_context(tc.tile_pool(name="ppp", bufs=2, space="PSUM"))
        pna0 = top.enter_context(tc.tile_pool(name="pna0", bufs=2, space="PSUM"))
        pna1 = top.enter_context(tc.tile_pool(name="pna1", bufs=1, space="PSUM"))

        def wload(name, shape, dtype=f32):
            t = wpool.tile(list(shape), dtype, tag=name, name=name)
            nc.sync.dma_start(t[:], P[name][:])
            return t

        w0comb_t = wload("w0comb", [108, 768], bf16)
        lvic_t = wload("lvic", [75, 32], bf16)
        vicb_t = wload("vicb", [32, 1])
        fp8 = mybir.dt.float8e4
        w1dr_t = wload("w1dr", [128, 2 * 768], fp8)
        w1dr3 = w1dr_t[:].rearrange("p (i m) -> p i m", i=2)
        b1_t = wload("b1t", [128, 6])
        lnfdr_t = wload("lnfdr", [128, 2 * 512], fp8)
        lnfdr3 = lnfdr_t[:].rearrange("p (i m) -> p i m", i=2)
        lhh0_t = wload("lhh0", [128, 512], bf16)
        lih1_t = wload("lih1", [128, 512], bf16)
        lhh1_t = wload("lhh1", [128, 512], bf16)
        ident_t = wload("ident", [128, 128], bf16)
        nb1bc_t = wload("nb1bc", [128, 512], bf16)
        outWT_t = wload("outWT", [128, 3], bf16)

        # persistent tiles
        xt = persist.tile([108, R], bf16, tag="xt")
        h1All = persist.tile([128, R], bf16, tag="h1All")
        pre0 = persist.tile([128, NN * 512], bf16, tag="pre0")
        ysb = persist.tile([3, R], bf16, tag="ysb")

        nc.sync.dma_start(xt[32:48, :], P["beat_bc"][:])
        nc.sync.dma_start(xt[48:96, :], P["e48"][:])
        nc.sync.dma_start(xt[96:108, :], P["chord_bc"][:])

        pre0_r = pre0[:].rearrange("p (n q b) -> p n q b", q=4, b=B)

        # ---- TA block emitters -----------------------------------------
        h0cur = {}
        nfcur = {}

        def ta_conv(blk):
            sl = slice(blk * BLK, (blk + 1) * BLK)
            im_t = im_pool.tile([75, BLK], bf16, tag="imblk", name="imblk")
            nc.sync.dma_start(im_t[:], P["im2colT"][:, sl])
            vps = pta.tile([32, BLK], f32, tag="pg", name="vps")
            nc.tensor.matmul(vps[:], lvic_t[:], im_t[:])
            nc.scalar.activation(xt[0:32, sl], vps[:], AF.Tanh,
                                 bias=vicb_t[:, 0:1])

        def ta_l0_half(blk, half):
            sl = slice(blk * BLK, (blk + 1) * BLK)
            pio = pta.tile([128, 2 * BLK], f32, tag="pio", name="pio")
            pg = pta.tile([128, BLK], f32, tag="pg", name="pg")
            nc.tensor.matmul(pio[:, 0:BLK],
                             w0comb_t[:, half * 128:(half + 1) * 128],
                             xt[:, sl])
            nc.tensor.matmul(pio[:, BLK:2 * BLK],
                             w0comb_t[:, (4 + half) * 128:(5 + half) * 128],
                             xt[:, sl])
            nc.tensor.matmul(pg[:],
                             w0comb_t[:, (2 + half) * 128:(3 + half) * 128],
                             xt[:, sl])
            sio = scr.tile([128, 2 * BLK], f32, tag="sio")
            nc.scalar.activation(sio[:, 0:BLK], pio[:, 0:BLK], AF.Sigmoid)
            nc.scalar.activation(sio[:, BLK:2 * BLK], pio[:, BLK:2 * BLK],
                                 AF.Sigmoid)
            tg = scr.tile([128, BLK], f32, tag="tg")
            nc.scalar.activation(tg[:], pg[:], AF.Tanh)
            c2 = scr.tile([128, BLK], f32, tag="c2")
            nc.gpsimd.tensor_tensor(c2[:], sio[:, 0:BLK], tg[:], Alu.mult)
            tc2 = scr.tile([128, BLK], f32, tag="tc2")
            nc.scalar.activation(tc2[:], c2[:], AF.Tanh)
            if half == 0:
                h0cur[0] = h0pool.tile([128, 2 * BLK], fp8, tag="h0T",
                                       name="h0T")
            h0t = h0cur[0]
            nc.vector.tensor_tensor(h0t[:, half * BLK:(half + 1) * BLK],
                                    sio[:, BLK:2 * BLK], tc2[:], Alu.mult)

        def ta_l1_half(blk, half):
            pio = pta.tile([128, 2 * BLK], f32, tag="pio", name="bpio")
            pg = pta.tile([128, BLK], f32, tag="pg", name="bpg")
            h0r3 = h0cur[0][:].rearrange("p (i n) -> p i n", i=2)
            from concourse import mybir as _mb
            dr = _mb.MatmulPerfMode.DoubleRow
            for q, cols in ((half, slice(0, BLK)),
                            (4 + half, slice(BLK, 2 * BLK))):
                qs = slice(q * 128, (q + 1) * 128)
                nc.tensor.matmul(pio[:, cols], w1dr3[:, :, qs], h0r3,
                                 perf_mode=dr, start=True, stop=True)
            qs = slice((2 + half) * 128, (3 + half) * 128)
            nc.tensor.matmul(pg[:], w1dr3[:, :, qs], h0r3,
                             perf_mode=dr, start=True, stop=True)
            sio = scr.tile([128, 2 * BLK], f32, tag="bsio")
            nc.scalar.activation(sio[:, 0:BLK], pio[:, 0:BLK], AF.Sigmoid,
                                 bias=b1_t[:, half:half + 1])
            nc.scalar.activation(sio[:, BLK:2 * BLK], pio[:, BLK:2 * BLK],
                                 AF.Sigmoid, bias=b1_t[:, 4 + half:5 + half])
            tg = scr.tile([128, BLK], f32, tag="btg")
            nc.scalar.activation(tg[:], pg[:], AF.Tanh,
                                 bias=b1_t[:, 2 + half:3 + half])
            c2 = scr.tile([128, BLK], f32, tag="bc2")
            nc.gpsimd.tensor_tensor(c2[:], sio[:, 0:BLK], tg[:], Alu.mult)
            tc2 = scr.tile([128, BLK], f32, tag="btc2")
            nc.scalar.activation(tc2[:], c2[:], AF.Tanh)
            if half == 0:
                nfcur[0] = nfpool.tile([128, 2 * BLK], fp8, tag="nfT",
                                       name="nfT")
            nft = nfcur[0]
            nc.vector.tensor_tensor(nft[:, half * BLK:(half + 1) * BLK],
                                    sio[:, BLK:2 * BLK], tc2[:], Alu.mult)

        def pre0_block(blk):
            """pre0[:, blk notes] = lnf0@nf0 + lnf1@nf1 (psum) + presh."""
            psh = im_pool.tile([128, 4 * 512], bf16, tag="psh", name="psh")
            nc.sync.dma_start(psh[:], P["presh"][:, blk * 2048:(blk + 1) * 2048])
            psh_r = psh[:].rearrange("p (n q b) -> p n q b", q=4, b=B)
            nf3 = nfcur[0][:].rearrange("p (i n) -> p i n", i=2)
            from concourse import mybir as _mb
            dr = _mb.MatmulPerfMode.DoubleRow
            for q in range(4):
                qs = slice(q * 128, (q + 1) * 128)
                pp = ppp.tile([128, BLK], f32, tag="pp", name="pp")
                nc.tensor.matmul(pp[:], lnfdr3[:, :, qs], nf3,
                                 perf_mode=dr, start=True, stop=True)
                nsl = slice(4 * blk, 4 * blk + 4)
                pp_r = pp[:].rearrange("p (n b) -> p n b", b=B)
                nc.vector.tensor_tensor(pre0_r[:, nsl, q, :], pp_r,
                                        psh_r[:, :, q, :], Alu.add)

        # ---- NoteAxis --------------------------------------------------
        c_prev = [None, None]
        na_ps0 = {}
        h0_ring = {}

        def open_ps0(n):
            ps0 = pna0.tile([128, 512], f32, tag="na0", name="ps0")
            nc.tensor.matmul(ps0[:], ident_t[:],
                             pre0[:, n * 512:(n + 1) * 512],
                             start=True, stop=(n == 0))
            na_ps0[n] = ps0

        def na_step(n):
            ns = slice(n * B, (n + 1) * B)
            pns = slice((n - 1) * B, n * B)
            # ps1 openers: bias preload + h1 recurrence
            ps1 = pna1.tile([128, 512], f32, tag="na1", name="ps1")
            nc.tensor.matmul(ps1[:], ident_t[:], nb1bc_t[:],
                             start=True, stop=False)
            if n > 0:
                for q in range(4):
                    qs = slice(q * 128, (q + 1) * 128)
                    nc.tensor.matmul(ps1[:, qs], lhh1_t[:, qs],
                                     h1All[:, pns], start=False, stop=False)
            # ps0 close: h0 recurrence (latency-critical chain)
            hp = tc.high_priority(300)
            hp.__enter__()
            ps0 = na_ps0.pop(n)
            if n > 0:
                h0p = h0_ring.pop(n - 1)
                for q in range(4):
                    qs = slice(q * 128, (q + 1) * 128)
                    nc.tensor.matmul(ps0[:, qs], lhh0_t[:, qs],
                                     h0p[:], start=False, stop=(q == 3))
            h0r = h0ring.tile([128, B], bf16, tag="h0r", name="h0r")
            h0_ring[n] = h0r
            c_prev[0] = _lstm_nl(nc, nascr, cpool, mybir, ps0,
                                 c_prev[0], h0r[:], tag="L0")
            for q in range(4):
                qs = slice(q * 128, (q + 1) * 128)
                nc.tensor.matmul(ps1[:, qs], lih1_t[:, qs], h0r[:],
                                 start=False, stop=(q == 3))
            c_prev[1] = _lstm_nl(nc, nascr, cpool, mybir, ps1,
                                 c_prev[1], h1All[:, ns], tag="L1")
            hp.__exit__(None, None, None)
            if n + 1 < NN:
                open_ps0(n + 1)

        def out_block(blk):
            sl = slice(blk * BLK, (blk + 1) * BLK)
            pox = pta.tile([3, BLK], f32, tag="pg", name="pox")
            nc.tensor.matmul(pox[:], outWT_t[:], h1All[:, sl])
            nc.vector.tensor_copy(ysb[:, sl], pox[:])

        # ---- interleaved pipeline: TA(blk) runs 3 blocks ahead of NA ---
        for blk in range(NBLK + 3):
            chunks = [lambda b=blk: ta_l0_half(b, 0),
                      lambda b=blk: ta_l0_half(b, 1),
                      lambda b=blk: ta_l1_half(b, 0),
                      lambda b=blk: ta_l1_half(b, 1)]
            if blk < NBLK:
                ta_conv(blk)
            for j in range(4):
                if blk < NBLK:
                    chunks[j]()
                if blk >= 3:
                    na_step(4 * (blk - 3) + j)
            if blk < NBLK:
                pre0_block(blk)
            if blk == 0:
                open_ps0(0)
        for blk in range(NBLK):
            out_block(blk)
        nc.sync.dma_start(P["y3"][:], ysb[:])
        if _DBG:
            nc.sync.dma_start(P["d_h1"][:], h1All[:])
            nc.sync.dma_start(P["d_pre0"][:], pre0[:])


def _lstm_nl(nc, scr, cpool, mybir, ps, c_prev, h_out, tag, dbg=None):
    """Gate nonlinearity + state update for one NoteAxis layer-step.

    One sigmoid covers all four gate blocks (i, f, g, o); tanh(g) is
    recovered as 2*sigmoid(2g)-1 with g-gate rows pre-doubled on the host.
    sf*c_prev runs on GPSIMD off the critical path.  Returns the new c tile.
    """
    f32 = mybir.dt.float32
    AF = mybir.ActivationFunctionType
    Alu = mybir.AluOpType

    s = scr.tile([128, 512], f32, tag=f"{tag}s")
    nc.scalar.activation(s[:], ps[:], AF.Sigmoid)
    if dbg is not None:
        dparam, dn = dbg
        nc.sync.dma_start(dparam[:, 512 * dn:512 * (dn + 1)], s[:])
    si, sf, sg, so = (s[:, 128 * k:128 * (k + 1)] for k in range(4))
    gt = scr.tile([128, 128], f32, tag=f"{tag}gt")
    nc.vector.tensor_scalar(gt[:], sg, 2.0, -1.0, Alu.mult, Alu.add)

    c_new = cpool.tile([128, 128], f32, tag=f"{tag}c")
    if c_prev is None:
        nc.vector.tensor_tensor(c_new[:], si, gt[:], Alu.mult)
    else:
        t2 = scr.tile([128, 128], f32, tag=f"{tag}t2")
        nc.gpsimd.tensor_tensor(t2[:], sf, c_prev[:], Alu.mult)
        t1 = scr.tile([128, 128], f32, tag=f"{tag}t1")
        nc.vector.tensor_tensor(t1[:], si, gt[:], Alu.mult)
        nc.vector.tensor_tensor(c_new[:], t1[:], t2[:], Alu.add)
    tcn = scr.tile([128, 128], f32, tag=f"{tag}tc")
    nc.scalar.activation(tcn[:], c_new[:], AF.Tanh)
    nc.vector.tensor_tensor(h_out, so, tcn[:], Alu.mult)
    return c_new


# --------------------------------------------------------------------------
# host side
# --------------------------------------------------------------------------

def _host_prep_weights(inp):
    import ml_dtypes

    f32 = np.float32
    bf16 = ml_dtypes.bfloat16

    W0 = np.asarray(inp["ta_Wih0"], f32)          # [1024, 73]
    sel = np.r_[0:256, 512:768, 768:1024]
    W0s = W0[sel]                                  # [768, 73] rows i,g,o
    b0s = (np.asarray(inp["ta_bih0"], f32) + np.asarray(inp["ta_bhh0"], f32))[sel]

    n = np.arange(NN)
    const_feat = np.zeros((13, NN), f32)
    const_feat[0] = n / NN
    const_feat[1 + (n % OCT), n] = 1.0

    beat_W = np.asarray(inp["beat_W"], f32)        # [16, 16]
    beat_b = np.asarray(inp["beat_b"], f32)
    gn = (W0s[:, 0:13] @ const_feat
          + (b0s + W0s[:, 13:29] @ beat_b)[:, None])        # [768, 48]
    Wbeat = W0s[:, 13:29] @ beat_W                 # [768, 16]
    Wvic = W0s[:, 29:61]                           # [768, 32]
    Wchord = W0s[:, 61:73]                         # [768, 12]
    w0comb = np.concatenate(
        [Wvic.T, Wbeat.T, gn.T, Wchord.T], axis=0
    ).astype(f32)                                  # [108, 768]

    vic_W = np.asarray(inp["vic_W"], f32)          # [32, 3, 25]
    lvic = vic_W.reshape(32, 75).T.copy()          # [75, 32] rows (c*25+s)
    vicb = np.asarray(inp["vic_b"], f32).reshape(32, 1)

    W1 = np.asarray(inp["ta_Wih1"], f32)[sel]      # [768, 256]
    b1s = (np.asarray(inp["ta_bih1"], f32) + np.asarray(inp["ta_bhh1"], f32))[sel]
    w1T = W1.T.astype(f32)                         # [256, 768]
    b1t = b1s.reshape(6, 128).T.copy()             # [128, 6]

    # sigma-trick: tanh(g) = 2*sigmoid(2g)-1, so double every g-gate row
    # (cols 256:384 of the transposed layouts) including the bias.
    def dbl_g(wT):
        wT = wT.copy()
        wT[:, 256:384] *= 2.0
        return wT

    naW0 = np.asarray(inp["na_Wih0"], f32)         # [512, 259]
    lnf = dbl_g(naW0[:, 0:256].T).astype(bf16)     # [256, 512]
    lhh0 = dbl_g(np.asarray(inp["na_Whh0"], f32).T).astype(bf16)
    lih1 = dbl_g(np.asarray(inp["na_Wih1"], f32).T).astype(bf16)
    lhh1 = dbl_g(np.asarray(inp["na_Whh1"], f32).T).astype(bf16)
    nb1 = (np.asarray(inp["na_bih1"], f32) + np.asarray(inp["na_bhh1"], f32))
    nb1d = dbl_g(nb1[None, :])[0]                  # [512]
    # nb1bc[p, q*128+b] = nb1d[q*128+p]
    nb1bc = np.broadcast_to(
        nb1d.reshape(4, 128).T[:, :, None], (128, 4, 128)
    ).reshape(128, 512).astype(bf16)

    outWT = np.asarray(inp["out_W"], f32).T.astype(bf16)     # [128, 3]

    return {
        "w0comb": w0comb.astype(bf16), "lvic": lvic.astype(bf16),
        "vicb": vicb,
        "w1dr": np.ascontiguousarray(
            w1T.reshape(2, 128, 768).transpose(1, 0, 2)
        ).reshape(128, 2 * 768).astype(ml_dtypes.float8_e4m3fn),
        "b1t": b1t,
        "lnfdr": np.ascontiguousarray(
            np.asarray(lnf, f32).reshape(2, 128, 512).transpose(1, 0, 2)
        ).reshape(128, 2 * 512).astype(ml_dtypes.float8_e4m3fn),
        "lhh0": lhh0, "lih1": lih1, "lhh1": lhh1,
        "ident": np.eye(128, dtype=f32).astype(bf16), "nb1bc": nb1bc,
        "outWT": outWT,
    }


def _host_prep_core(inp, note, beat, cond):
    """Per-core input gathering. note [B,48,3] etc."""
    import ml_dtypes

    f32 = np.float32
    bf16 = ml_dtypes.bfloat16
    pn = np.zeros((B, 72, 3), f32)
    pn[:, 12:60, :] = note
    # im2colT[(c*25+s), (n, b)] = pn[b, n+s, c]
    win = np.stack([pn[:, s:s + 48, :] for s in range(25)], axis=0)  # [25,B,48,3]
    im2colT = np.ascontiguousarray(win.transpose(3, 0, 2, 1)).reshape(75, R)

    beat_bc = np.ascontiguousarray(
        np.broadcast_to(beat.T[:, None, :], (16, NN, B))
    ).reshape(16, R)
    e48 = np.repeat(np.eye(48, dtype=f32), B, axis=1)        # [48, R]
    chord = (note[:, :, 0] / 4.0).reshape(B, OCT, 4).sum(axis=2)  # [B, 12]
    chord_bc = np.ascontiguousarray(
        np.broadcast_to(chord.T[:, None, :], (12, NN, B))
    ).reshape(12, R)

    sh = np.zeros((B, NN, 3), f32)
    sh[:, 1:, :] = cond[:, :-1, :]
    # presh[u,(n,b)] = Wsh @ shifted + bias0, then g-rows doubled,
    # laid out [p, (n, q, b)] with p = unit-within-chunk.
    naW0 = np.asarray(inp["na_Wih0"], f32)
    Wsh = naW0[:, 256:259]                          # [512, 3]
    nb0 = (np.asarray(inp["na_bih0"], f32) + np.asarray(inp["na_bhh0"], f32))
    G = np.einsum("uk,bnk->unb", Wsh, sh) + nb0[:, None, None]  # [512,n,b]
    G[256:384] *= 2.0
    presh = np.ascontiguousarray(
        G.reshape(4, 128, NN, B).transpose(1, 2, 0, 3)
    ).reshape(128, NN * 512).astype(bf16)

    return {
        "im2colT": im2colT.astype(bf16), "beat_bc": beat_bc.astype(bf16),
        "e48": e48.astype(bf16), "chord_bc": chord_bc.astype(bf16),
        "presh": presh,
    }


def _host_finish(y3, out_b):
    """y3 [3, R] raw out-projection -> [B, 48, 3] with bias + sigmoid."""
    v = y3.astype(np.float64).reshape(3, NN, B).transpose(2, 1, 0)
    v = v + out_b[None, None, :]
    v[:, :, 0:2] = 1.0 / (1.0 + np.exp(-v[:, :, 0:2]))
    return v.astype(np.float32)


def kernel(**inputs):
    from concourse.bass_utils import run_bass_kernel_spmd

    if "prog" not in _PROGRAM_CACHE:
        _PROGRAM_CACHE["prog"] = _build_program()
    nc = _PROGRAM_CACHE["prog"]

    wmap = _host_prep_weights(inputs)
    note = np.asarray(inputs["note_input"], np.float32)
    beat = np.asarray(inputs["beat_in"], np.float32)
    cond = np.asarray(inputs["condition_notes"], np.float32)
    out_b = np.asarray(inputs["out_b"], np.float64)

    in_maps = []
    for c in range(N_CORES):
        bs = slice(c * B, (c + 1) * B)
        m = dict(wmap)
        m.update(_host_prep_core(inputs, note[bs], beat[bs], cond[bs]))
        in_maps.append(m)

    res = run_bass_kernel_spmd(nc, in_maps, list(range(N_CORES)))
    outs = [_host_finish(res.results[c]["y3"], out_b) for c in range(N_CORES)]
    return np.concatenate(outs, axis=0).astype(np.float32)


# revision 25
# speedup vs baseline: 1.0044x; 1.0044x over previous
"""DeepJ (TimeAxis + NoteAxis LSTM) Trainium2 kernel, v2.

Data-parallel over 8 NeuronCores: batch 1024 -> 128 per core.

Layout: activations are [units, rows] tiles with rows = (note, batch) on the
free dimension; weights are the stationary (lhsT) matmul operands, so the
NoteAxis recurrence needs no per-step transposes.

v2 structure (vs v1):
  * NoteAxis input projections (Wih0 @ [nf; shifted] + bias) are batched into
    the feed-forward phase as N=512 matmuls and stored in SBUF (pre0, bf16).
    Each NA step loads them into PSUM with ONE identity-matmul that also
    opens the bank's single accumulation group -- the v1 bug was four
    interleaved per-chunk groups in one bank: start=True clears the whole
    bank's has_written bits, so later start=False closers overwrote the
    openers' data instead of accumulating.
  * The K=4 shifted-notes projection and the chord broadcast are computed on
    the host (numpy); output bias + sigmoid applied on host as well.
  * Output projection is 12 [M=3, N=512] matmuls instead of 48 [*, N=3].
  * h0/nf TA tiles are per-block rings instead of full-R persists.

Matmul dtype is bfloat16 throughout (f32r lowers to 2x hi/lo passes
on this target, doubling PE time for no useful accuracy here).
"""

import os
import sys

for _p in ("/opt/trn_rl_repo",):
    if _p not in sys.path:
        sys.path.insert(0, _p)

import numpy as np

# ---- model constants -------------------------------------------------------
N_CORES = 8
B_TOT = 1024
B = B_TOT // N_CORES          # 128 rows per core
NN = 48                       # notes
OCT = 12
R = NN * B                    # 6144 rows, ordered (note, batch)
NBLK = 12                     # row blocks of 512 for the feed-forward stages
BLK = 512

_PROGRAM_CACHE = {}
_DBG = bool(os.environ.get("DEEPJ_DEBUG"))


def _build_program():
    import concourse.tile as tile
    from concourse import bacc, mybir

    f32 = mybir.dt.float32
    f32r = mybir.dt.float32r
    bf16 = mybir.dt.bfloat16

    nc = bacc.Bacc(
        "TRN2", target_bir_lowering=False, debug=False, num_devices=N_CORES
    )

    def param(name, shape, dtype=f32):
        return nc.declare_dram_parameter(name, list(shape), dtype, isOutput=False)

    P = {}
    # per-core activations / gathered inputs
    P["im2colT"] = param("im2colT", [75, R], bf16)   # conv patches, (c*25+s, (n,b))
    P["beat_bc"] = param("beat_bc", [16, R], bf16)   # beat_in^T bcast over n
    P["e48"] = param("e48", [48, R], bf16)           # one-hot(n) bcast over b
    P["chord_bc"] = param("chord_bc", [12, R], bf16)  # chord^T bcast over n
    P["presh"] = param("presh", [128, NN * 512], bf16)  # Wsh@shifted+b0, (n,q,b)
    # weights (replicated on every core)
    P["w0comb"] = param("w0comb", [108, 768], bf16)  # folded TA-L0 lhsT
    P["lvic"] = param("lvic", [75, 32], bf16)        # conv lhsT
    P["vicb"] = param("vicb", [32, 1])
    P["w1dr"] = param("w1dr", [128, 2 * 768], mybir.dt.float8e4)  # TA-L1 DoubleRow lhsT
    P["b1t"] = param("b1t", [128, 6])                # TA-L1 bias per u-chunk
    P["lnfdr"] = param("lnfdr", [128, 2 * 512], mybir.dt.float8e4)  # NA-L0 Wih DoubleRow lhsT
    P["lhh0"] = param("lhh0", [128, 512], bf16)      # NA-L0 Whh lhsT
    P["lih1"] = param("lih1", [128, 512], bf16)      # NA-L1 Wih lhsT
    P["lhh1"] = param("lhh1", [128, 512], bf16)      # NA-L1 Whh lhsT
    P["ident"] = param("ident", [128, 128], bf16)    # identity (psum preload)
    P["nb1bc"] = param("nb1bc", [128, 512], bf16)    # NA-L1 bias, (q,b) bcast
    P["outWT"] = param("outWT", [128, 3], bf16)
    P["y3"] = nc.declare_dram_parameter("y3", [3, R], bf16, isOutput=True)
    if _DBG:
        for nm, shp, dt in [("d_h1", [128, R], bf16),
                            ("d_g0", [128, 512 * NN], f32),
                            ("d_s0", [128, 512 * NN], f32),
                            ("d_h0na", [128, B * NN], bf16),
                            ("d_pre0", [128, NN * 512], bf16)]:
            P[nm] = nc.declare_dram_parameter(nm, shp, dt, isOutput=True)

    with tile.TileContext(nc) as tc:
        _emit(nc, tc, mybir, P)
    nc.compile()
    return nc


def _emit(nc, tc, mybir, P):
    from contextlib import ExitStack

    f32 = mybir.dt.float32
    f32r = mybir.dt.float32r
    bf16 = mybir.dt.bfloat16
    AF = mybir.ActivationFunctionType
    Alu = mybir.AluOpType

    with ExitStack() as top:
        wpool = top.enter_context(tc.tile_pool(name="weights", bufs=1))
        persist = top.enter_context(tc.tile_pool(name="persist", bufs=1))
        h0pool = top.enter_context(tc.tile_pool(name="h0ring2", bufs=2))
        nfpool = top.enter_context(tc.tile_pool(name="nfring", bufs=2))
        scr = top.enter_context(tc.tile_pool(name="scr", bufs=2))
        nascr = top.enter_context(tc.tile_pool(name="nascr", bufs=3))
        h0ring = top.enter_context(tc.tile_pool(name="h0ring", bufs=3))
        cpool = top.enter_context(tc.tile_pool(name="cstate", bufs=2))
        im_pool = top.enter_context(tc.tile_pool(name="im", bufs=3))
        pta = top.enter_context(tc.tile_pool(name="pta", bufs=1, space="PSUM"))
        ppp = top.enter_context(tc.tile_pool(name="ppp", bufs=2, space="PSUM"))
        pna0 = top.enter_context(tc.tile_pool(name="pna0", bufs=2, space="PSUM"))
        pna1 = top.enter_context(tc.tile_pool(name="pna1", bufs=1, space="PSUM"))

        def wload(name, shape, dtype=f32):
            t = wpool.tile(list(shape), dtype, tag=name, name=name)
            nc.sync.dma_start(t[:], P[name][:])
            return t

        w0comb_t = wload("w0comb", [108, 768], bf16)
        lvic_t = wload("lvic", [75, 32], bf16)
        vicb_t = wload("vicb", [32, 1])
        fp8 = mybir.dt.float8e4
        w1dr_t = wload("w1dr", [128, 2 * 768], fp8)
        w1dr3 = w1dr_t[:].rearrange("p (i m) -> p i m", i=2)
        b1_t = wload("b1t", [128, 6])
        lnfdr_t = wload("lnfdr", [128, 2 * 512], fp8)
        lnfdr3 = lnfdr_t[:].rearrange("p (i m) -> p i m", i=2)
        lhh0_t = wload("lhh0", [128, 512], bf16)
        lih1_t = wload("lih1", [128, 512], bf16)
        lhh1_t = wload("lhh1", [128, 512], bf16)
        ident_t = wload("ident", [128, 128], bf16)
        nb1bc_t = wload("nb1bc", [128, 512], bf16)
        outWT_t = wload("outWT", [128, 3], bf16)

        # persistent tiles
        xt = persist.tile([108, R], bf16, tag="xt")
        h1All = persist.tile([128, R], bf16, tag="h1All")
        pre0 = persist.tile([128, NN * 512], bf16, tag="pre0")
        ysb = persist.tile([3, R], bf16, tag="ysb")

        nc.sync.dma_start(xt[32:48, :], P["beat_bc"][:])
        nc.sync.dma_start(xt[48:96, :], P["e48"][:])
        nc.sync.dma_start(xt[96:108, :], P["chord_bc"][:])

        pre0_r = pre0[:].rearrange("p (n q b) -> p n q b", q=4, b=B)

        # ---- TA block emitters -----------------------------------------
        h0cur = {}
        nfcur = {}

        def ta_conv(blk):
            sl = slice(blk * BLK, (blk + 1) * BLK)
            im_t = im_pool.tile([75, BLK], bf16, tag="imblk", name="imblk")
            nc.sync.dma_start(im_t[:], P["im2colT"][:, sl])
            vps = pta.tile([32, BLK], f32, tag="pg", name="vps")
            nc.tensor.matmul(vps[:], lvic_t[:], im_t[:])
            nc.scalar.activation(xt[0:32, sl], vps[:], AF.Tanh,
                                 bias=vicb_t[:, 0:1])

        def ta_l0_half(blk, half):
            sl = slice(blk * BLK, (blk + 1) * BLK)
            pio = pta.tile([128, 2 * BLK], f32, tag="pio", name="pio")
            pg = pta.tile([128, BLK], f32, tag="pg", name="pg")
            nc.tensor.matmul(pio[:, 0:BLK],
                             w0comb_t[:, half * 128:(half + 1) * 128],
                             xt[:, sl])
            nc.tensor.matmul(pio[:, BLK:2 * BLK],
                             w0comb_t[:, (4 + half) * 128:(5 + half) * 128],
                             xt[:, sl])
            nc.tensor.matmul(pg[:],
                             w0comb_t[:, (2 + half) * 128:(3 + half) * 128],
                             xt[:, sl])
            sio = scr.tile([128, 2 * BLK], f32, tag="sio")
            nc.scalar.activation(sio[:, 0:BLK], pio[:, 0:BLK], AF.Sigmoid)
            nc.scalar.activation(sio[:, BLK:2 * BLK], pio[:, BLK:2 * BLK],
                                 AF.Sigmoid)
            tg = scr.tile([128, BLK], f32, tag="tg")
            nc.scalar.activation(tg[:], pg[:], AF.Tanh)
            c2 = scr.tile([128, BLK], f32, tag="c2")
            nc.gpsimd.tensor_tensor(c2[:], sio[:, 0:BLK], tg[:], Alu.mult)
            tc2 = scr.tile([128, BLK], f32, tag="tc2")
            nc.scalar.activation(tc2[:], c2[:], AF.Tanh)
            if half == 0:
                h0cur[0] = h0pool.tile([128, 2 * BLK], fp8, tag="h0T",
                                       name="h0T")
            h0t = h0cur[0]
            hb = half * BLK
            for o in (0, BLK // 2):
                nc.vector.tensor_tensor(h0t[:, hb + o:hb + o + BLK // 2],
                                        sio[:, BLK + o:BLK + o + BLK // 2],
                                        tc2[:, o:o + BLK // 2], Alu.mult)

        def ta_l1_half(blk, half):
            pio = pta.tile([128, 2 * BLK], f32, tag="pio", name="bpio")
            pg = pta.tile([128, BLK], f32, tag="pg", name="bpg")
            h0r3 = h0cur[0][:].rearrange("p (i n) -> p i n", i=2)
            from concourse import mybir as _mb
            dr = _mb.MatmulPerfMode.DoubleRow
            for q, cols in ((half, slice(0, BLK)),
                            (4 + half, slice(BLK, 2 * BLK))):
                qs = slice(q * 128, (q + 1) * 128)
                nc.tensor.matmul(pio[:, cols], w1dr3[:, :, qs], h0r3,
                                 perf_mode=dr, start=True, stop=True)
            qs = slice((2 + half) * 128, (3 + half) * 128)
            nc.tensor.matmul(pg[:], w1dr3[:, :, qs], h0r3,
                             perf_mode=dr, start=True, stop=True)
            sio = scr.tile([128, 2 * BLK], f32, tag="bsio")
            nc.scalar.activation(sio[:, 0:BLK], pio[:, 0:BLK], AF.Sigmoid,
                                 bias=b1_t[:, half:half + 1])
            nc.scalar.activation(sio[:, BLK:2 * BLK], pio[:, BLK:2 * BLK],
                                 AF.Sigmoid, bias=b1_t[:, 4 + half:5 + half])
            tg = scr.tile([128, BLK], f32, tag="btg")
            nc.scalar.activation(tg[:], pg[:], AF.Tanh,
                                 bias=b1_t[:, 2 + half:3 + half])
            c2 = scr.tile([128, BLK], f32, tag="bc2")
            nc.gpsimd.tensor_tensor(c2[:], sio[:, 0:BLK], tg[:], Alu.mult)
            tc2 = scr.tile([128, BLK], f32, tag="btc2")
            nc.scalar.activation(tc2[:], c2[:], AF.Tanh)
            if half == 0:
                nfcur[0] = nfpool.tile([128, 2 * BLK], fp8, tag="nfT",
                                       name="nfT")
            nft = nfcur[0]
            hb = half * BLK
            for o in (0, BLK // 2):
                nc.vector.tensor_tensor(nft[:, hb + o:hb + o + BLK // 2],
                                        sio[:, BLK + o:BLK + o + BLK // 2],
                                        tc2[:, o:o + BLK // 2], Alu.mult)

        def pre0_block(blk):
            """pre0[:, blk notes] = lnf0@nf0 + lnf1@nf1 (psum) + presh."""
            psh = im_pool.tile([128, 4 * 512], bf16, tag="psh", name="psh")
            nc.sync.dma_start(psh[:], P["presh"][:, blk * 2048:(blk + 1) * 2048])
            psh_r = psh[:].rearrange("p (n q b) -> p n q b", q=4, b=B)
            nf3 = nfcur[0][:].rearrange("p (i n) -> p i n", i=2)
            from concourse import mybir as _mb
            dr = _mb.MatmulPerfMode.DoubleRow
            for q in range(4):
                qs = slice(q * 128, (q + 1) * 128)
                pp = ppp.tile([128, BLK], f32, tag="pp", name="pp")
                nc.tensor.matmul(pp[:], lnfdr3[:, :, qs], nf3,
                                 perf_mode=dr, start=True, stop=True)
                pp_r = pp[:].rearrange("p (n b) -> p n b", b=B)
                for o in (0, 2):
                    nsl = slice(4 * blk + o, 4 * blk + o + 2)
                    nc.vector.tensor_tensor(pre0_r[:, nsl, q, :],
                                            pp_r[:, o:o + 2, :],
                                            psh_r[:, o:o + 2, q, :], Alu.add)

        # ---- NoteAxis --------------------------------------------------
        c_prev = [None, None]
        na_ps0 = {}
        h0_ring = {}

        def open_ps0(n):
            ps0 = pna0.tile([128, 512], f32, tag="na0", name="ps0")
            nc.tensor.matmul(ps0[:], ident_t[:],
                             pre0[:, n * 512:(n + 1) * 512],
                             start=True, stop=(n == 0))
            na_ps0[n] = ps0

        def na_step(n):
            ns = slice(n * B, (n + 1) * B)
            pns = slice((n - 1) * B, n * B)
            # ps1 openers: bias preload + h1 recurrence
            ps1 = pna1.tile([128, 512], f32, tag="na1", name="ps1")
            nc.tensor.matmul(ps1[:], ident_t[:], nb1bc_t[:],
                             start=True, stop=False)
            if n > 0:
                for q in range(4):
                    qs = slice(q * 128, (q + 1) * 128)
                    nc.tensor.matmul(ps1[:, qs], lhh1_t[:, qs],
                                     h1All[:, pns], start=False, stop=False)
            # ps0 close: h0 recurrence (latency-critical chain)
            hp = tc.high_priority(300)
            hp.__enter__()
            ps0 = na_ps0.pop(n)
            if n > 0:
                h0p = h0_ring.pop(n - 1)
                for q in range(4):
                    qs = slice(q * 128, (q + 1) * 128)
                    nc.tensor.matmul(ps0[:, qs], lhh0_t[:, qs],
                                     h0p[:], start=False, stop=(q == 3))
            h0r = h0ring.tile([128, B], bf16, tag="h0r", name="h0r")
            h0_ring[n] = h0r
            c_prev[0] = _lstm_nl(nc, nascr, cpool, mybir, ps0,
                                 c_prev[0], h0r[:], tag="L0")
            for q in range(4):
                qs = slice(q * 128, (q + 1) * 128)
                nc.tensor.matmul(ps1[:, qs], lih1_t[:, qs], h0r[:],
                                 start=False, stop=(q == 3))
            c_prev[1] = _lstm_nl(nc, nascr, cpool, mybir, ps1,
                                 c_prev[1], h1All[:, ns], tag="L1")
            hp.__exit__(None, None, None)
            if n + 1 < NN:
                open_ps0(n + 1)

        def out_block(blk):
            sl = slice(blk * BLK, (blk + 1) * BLK)
            pox = pta.tile([3, BLK], f32, tag="pg", name="pox")
            nc.tensor.matmul(pox[:], outWT_t[:], h1All[:, sl])
            nc.vector.tensor_copy(ysb[:, sl], pox[:])

        # ---- interleaved pipeline: TA(blk) runs 3 blocks ahead of NA ---
        for blk in range(NBLK + 3):
            chunks = [lambda b=blk: ta_l0_half(b, 0),
                      lambda b=blk: ta_l0_half(b, 1),
                      lambda b=blk: ta_l1_half(b, 0),
                      lambda b=blk: ta_l1_half(b, 1)]
            if blk < NBLK:
                ta_conv(blk)
            for j in range(4):
                if blk < NBLK:
                    chunks[j]()
                if blk >= 3:
                    na_step(4 * (blk - 3) + j)
            if blk < NBLK:
                pre0_block(blk)
            if blk == 0:
                open_ps0(0)
        for blk in range(NBLK):
            out_block(blk)
        nc.sync.dma_start(P["y3"][:], ysb[:])
        if _DBG:
            nc.sync.dma_start(P["d_h1"][:], h1All[:])
            nc.sync.dma_start(P["d_pre0"][:], pre0[:])


def _lstm_nl(nc, scr, cpool, mybir, ps, c_prev, h_out, tag, dbg=None):
    """Gate nonlinearity + state update for one NoteAxis layer-step.

    One sigmoid covers all four gate blocks (i, f, g, o); tanh(g) is
    recovered as 2*sigmoid(2g)-1 with g-gate rows pre-doubled on the host.
    sf*c_prev runs on GPSIMD off the critical path.  Returns the new c tile.
    """
    f32 = mybir.dt.float32
    AF = mybir.ActivationFunctionType
    Alu = mybir.AluOpType

    s = scr.tile([128, 512], f32, tag=f"{tag}s")
    nc.scalar.activation(s[:], ps[:], AF.Sigmoid)
    if dbg is not None:
        dparam, dn = dbg
        nc.sync.dma_start(dparam[:, 512 * dn:512 * (dn + 1)], s[:])
    si, sf, sg, so = (s[:, 128 * k:128 * (k + 1)] for k in range(4))
    gt = scr.tile([128, 128], f32, tag=f"{tag}gt")
    nc.vector.tensor_scalar(gt[:], sg, 2.0, -1.0, Alu.mult, Alu.add)

    c_new = cpool.tile([128, 128], f32, tag=f"{tag}c")
    if c_prev is None:
        nc.vector.tensor_tensor(c_new[:], si, gt[:], Alu.mult)
    else:
        t2 = scr.tile([128, 128], f32, tag=f"{tag}t2")
        nc.vector.tensor_tensor(t2[:], sf, c_prev[:], Alu.mult)
        t1 = scr.tile([128, 128], f32, tag=f"{tag}t1")
        nc.vector.tensor_tensor(t1[:], si, gt[:], Alu.mult)
        nc.vector.tensor_tensor(c_new[:], t1[:], t2[:], Alu.add)
    tcn = scr.tile([128, 128], f32, tag=f"{tag}tc")
    nc.scalar.activation(tcn[:], c_new[:], AF.Tanh)
    nc.vector.tensor_tensor(h_out, so, tcn[:], Alu.mult)
    return c_new


# --------------------------------------------------------------------------
# host side
# --------------------------------------------------------------------------

def _host_prep_weights(inp):
    import ml_dtypes

    f32 = np.float32
    bf16 = ml_dtypes.bfloat16

    W0 = np.asarray(inp["ta_Wih0"], f32)          # [1024, 73]
    sel = np.r_[0:256, 512:768, 768:1024]
    W0s = W0[sel]                                  # [768, 73] rows i,g,o
    b0s = (np.asarray(inp["ta_bih0"], f32) + np.asarray(inp["ta_bhh0"], f32))[sel]

    n = np.arange(NN)
    const_feat = np.zeros((13, NN), f32)
    const_feat[0] = n / NN
    const_feat[1 + (n % OCT), n] = 1.0

    beat_W = np.asarray(inp["beat_W"], f32)        # [16, 16]
    beat_b = np.asarray(inp["beat_b"], f32)
    gn = (W0s[:, 0:13] @ const_feat
          + (b0s + W0s[:, 13:29] @ beat_b)[:, None])        # [768, 48]
    Wbeat = W0s[:, 13:29] @ beat_W                 # [768, 16]
    Wvic = W0s[:, 29:61]                           # [768, 32]
    Wchord = W0s[:, 61:73]                         # [768, 12]
    w0comb = np.concatenate(
        [Wvic.T, Wbeat.T, gn.T, Wchord.T], axis=0
    ).astype(f32)                                  # [108, 768]

    vic_W = np.asarray(inp["vic_W"], f32)          # [32, 3, 25]
    lvic = vic_W.reshape(32, 75).T.copy()          # [75, 32] rows (c*25+s)
    vicb = np.asarray(inp["vic_b"], f32).reshape(32, 1)

    W1 = np.asarray(inp["ta_Wih1"], f32)[sel]      # [768, 256]
    b1s = (np.asarray(inp["ta_bih1"], f32) + np.asarray(inp["ta_bhh1"], f32))[sel]
    w1T = W1.T.astype(f32)                         # [256, 768]
    b1t = b1s.reshape(6, 128).T.copy()             # [128, 6]

    # sigma-trick: tanh(g) = 2*sigmoid(2g)-1, so double every g-gate row
    # (cols 256:384 of the transposed layouts) including the bias.
    def dbl_g(wT):
        wT = wT.copy()
        wT[:, 256:384] *= 2.0
        return wT

    naW0 = np.asarray(inp["na_Wih0"], f32)         # [512, 259]
    lnf = dbl_g(naW0[:, 0:256].T).astype(bf16)     # [256, 512]
    lhh0 = dbl_g(np.asarray(inp["na_Whh0"], f32).T).astype(bf16)
    lih1 = dbl_g(np.asarray(inp["na_Wih1"], f32).T).astype(bf16)
    lhh1 = dbl_g(np.asarray(inp["na_Whh1"], f32).T).astype(bf16)
    nb1 = (np.asarray(inp["na_bih1"], f32) + np.asarray(inp["na_bhh1"], f32))
    nb1d = dbl_g(nb1[None, :])[0]                  # [512]
    # nb1bc[p, q*128+b] = nb1d[q*128+p]
    nb1bc = np.broadcast_to(
        nb1d.reshape(4, 128).T[:, :, None], (128, 4, 128)
    ).reshape(128, 512).astype(bf16)

    outWT = np.asarray(inp["out_W"], f32).T.astype(bf16)     # [128, 3]

    return {
        "w0comb": w0comb.astype(bf16), "lvic": lvic.astype(bf16),
        "vicb": vicb,
        "w1dr": np.ascontiguousarray(
            w1T.reshape(2, 128, 768).transpose(1, 0, 2)
        ).reshape(128, 2 * 768).astype(ml_dtypes.float8_e4m3fn),
        "b1t": b1t,
        "lnfdr": np.ascontiguousarray(
            np.asarray(lnf, f32).reshape(2, 128, 512).transpose(1, 0, 2)
        ).reshape(128, 2 * 512).astype(ml_dtypes.float8_e4m3fn),
        "lhh0": lhh0, "lih1": lih1, "lhh1": lhh1,
        "ident": np.eye(128, dtype=f32).astype(bf16), "nb1bc": nb1bc,
        "outWT": outWT,
    }


def _host_prep_core(inp, note, beat, cond):
    """Per-core input gathering. note [B,48,3] etc."""
    import ml_dtypes

    f32 = np.float32
    bf16 = ml_dtypes.bfloat16
    pn = np.zeros((B, 72, 3), f32)
    pn[:, 12:60, :] = note
    # im2colT[(c*25+s), (n, b)] = pn[b, n+s, c]
    win = np.stack([pn[:, s:s + 48, :] for s in range(25)], axis=0)  # [25,B,48,3]
    im2colT = np.ascontiguousarray(win.transpose(3, 0, 2, 1)).reshape(75, R)

    beat_bc = np.ascontiguousarray(
        np.broadcast_to(beat.T[:, None, :], (16, NN, B))
    ).reshape(16, R)
    e48 = np.repeat(np.eye(48, dtype=f32), B, axis=1)        # [48, R]
    chord = (note[:, :, 0] / 4.0).reshape(B, OCT, 4).sum(axis=2)  # [B, 12]
    chord_bc = np.ascontiguousarray(
        np.broadcast_to(chord.T[:, None, :], (12, NN, B))
    ).reshape(12, R)

    sh = np.zeros((B, NN, 3), f32)
    sh[:, 1:, :] = cond[:, :-1, :]
    # presh[u,(n,b)] = Wsh @ shifted + bias0, then g-rows doubled,
    # laid out [p, (n, q, b)] with p = unit-within-chunk.
    naW0 = np.asarray(inp["na_Wih0"], f32)
    Wsh = naW0[:, 256:259]                          # [512, 3]
    nb0 = (np.asarray(inp["na_bih0"], f32) + np.asarray(inp["na_bhh0"], f32))
    G = np.einsum("uk,bnk->unb", Wsh, sh) + nb0[:, None, None]  # [512,n,b]
    G[256:384] *= 2.0
    presh = np.ascontiguousarray(
        G.reshape(4, 128, NN, B).transpose(1, 2, 0, 3)
    ).reshape(128, NN * 512).astype(bf16)

    return {
        "im2colT": im2colT.astype(bf16), "beat_bc": beat_bc.astype(bf16),
        "e48": e48.astype(bf16), "chord_bc": chord_bc.astype(bf16),
        "presh": presh,
    }


def _host_finish(y3, out_b):
    """y3 [3, R] raw out-projection -> [B, 48, 3] with bias + sigmoid."""
    v = y3.astype(np.float64).reshape(3, NN, B).transpose(2, 1, 0)
    v = v + out_b[None, None, :]
    v[:, :, 0:2] = 1.0 / (1.0 + np.exp(-v[:, :, 0:2]))
    return v.astype(np.float32)


def kernel(**inputs):
    from concourse.bass_utils import run_bass_kernel_spmd

    if "prog" not in _PROGRAM_CACHE:
        _PROGRAM_CACHE["prog"] = _build_program()
    nc = _PROGRAM_CACHE["prog"]

    wmap = _host_prep_weights(inputs)
    note = np.asarray(inputs["note_input"], np.float32)
    beat = np.asarray(inputs["beat_in"], np.float32)
    cond = np.asarray(inputs["condition_notes"], np.float32)
    out_b = np.asarray(inputs["out_b"], np.float64)

    in_maps = []
    for c in range(N_CORES):
        bs = slice(c * B, (c + 1) * B)
        m = dict(wmap)
        m.update(_host_prep_core(inputs, note[bs], beat[bs], cond[bs]))
        in_maps.append(m)

    res = run_bass_kernel_spmd(nc, in_maps, list(range(N_CORES)))
    outs = [_host_finish(res.results[c]["y3"], out_b) for c in range(N_CORES)]
    return np.concatenate(outs, axis=0).astype(np.float32)


# revision 26
# speedup vs baseline: 1.0508x; 1.0461x over previous
"""DeepJ (TimeAxis + NoteAxis LSTM) Trainium2 kernel, v2.

Data-parallel over 8 NeuronCores: batch 1024 -> 128 per core.

Layout: activations are [units, rows] tiles with rows = (note, batch) on the
free dimension; weights are the stationary (lhsT) matmul operands, so the
NoteAxis recurrence needs no per-step transposes.

v2 structure (vs v1):
  * NoteAxis input projections (Wih0 @ [nf; shifted] + bias) are batched into
    the feed-forward phase as N=512 matmuls and stored in SBUF (pre0, bf16).
    Each NA step loads them into PSUM with ONE identity-matmul that also
    opens the bank's single accumulation group -- the v1 bug was four
    interleaved per-chunk groups in one bank: start=True clears the whole
    bank's has_written bits, so later start=False closers overwrote the
    openers' data instead of accumulating.
  * The K=4 shifted-notes projection and the chord broadcast are computed on
    the host (numpy); output bias + sigmoid applied on host as well.
  * Output projection is 12 [M=3, N=512] matmuls instead of 48 [*, N=3].
  * h0/nf TA tiles are per-block rings instead of full-R persists.

Matmul dtype is bfloat16 throughout (f32r lowers to 2x hi/lo passes
on this target, doubling PE time for no useful accuracy here).
"""

import os
import sys

for _p in ("/opt/trn_rl_repo",):
    if _p not in sys.path:
        sys.path.insert(0, _p)

import numpy as np

# ---- model constants -------------------------------------------------------
N_CORES = 8
B_TOT = 1024
B = B_TOT // N_CORES          # 128 rows per core
NN = 48                       # notes
OCT = 12
R = NN * B                    # 6144 rows, ordered (note, batch)
NBLK = 12                     # row blocks of 512 for the feed-forward stages
BLK = 512

_PROGRAM_CACHE = {}
_DBG = bool(os.environ.get("DEEPJ_DEBUG"))


def _build_program():
    import concourse.tile as tile
    from concourse import bacc, mybir

    f32 = mybir.dt.float32
    f32r = mybir.dt.float32r
    bf16 = mybir.dt.bfloat16

    nc = bacc.Bacc(
        "TRN2", target_bir_lowering=False, debug=False, num_devices=N_CORES
    )

    def param(name, shape, dtype=f32):
        return nc.declare_dram_parameter(name, list(shape), dtype, isOutput=False)

    P = {}
    # per-core activations / gathered inputs
    P["im2colT"] = param("im2colT", [75, R], bf16)   # conv patches, (c*25+s, (n,b))
    P["beat_bc"] = param("beat_bc", [16, R], bf16)   # beat_in^T bcast over n
    P["e48"] = param("e48", [48, R], bf16)           # one-hot(n) bcast over b
    P["chord_bc"] = param("chord_bc", [12, R], bf16)  # chord^T bcast over n
    P["presh"] = param("presh", [128, NN * 512], bf16)  # Wsh@shifted+b0, (n,q,b)
    # weights (replicated on every core)
    P["w0comb"] = param("w0comb", [108, 768], bf16)  # folded TA-L0 lhsT
    P["lvic"] = param("lvic", [75, 32], bf16)        # conv lhsT
    P["vicb"] = param("vicb", [32, 1])
    P["w1dr"] = param("w1dr", [128, 2 * 768], mybir.dt.float8e4)  # TA-L1 DoubleRow lhsT
    P["b1t"] = param("b1t", [128, 6])                # TA-L1 bias per u-chunk
    P["lnfdr"] = param("lnfdr", [128, 2 * 512], mybir.dt.float8e4)  # NA-L0 Wih DoubleRow lhsT
    P["lhh0"] = param("lhh0", [128, 512], bf16)      # NA-L0 Whh lhsT
    P["lih1"] = param("lih1", [128, 512], bf16)      # NA-L1 Wih lhsT
    P["lhh1"] = param("lhh1", [128, 512], bf16)      # NA-L1 Whh lhsT
    P["ident"] = param("ident", [128, 128], bf16)    # identity (psum preload)
    P["nb1bc"] = param("nb1bc", [128, 512], bf16)    # NA-L1 bias, (q,b) bcast
    P["outWT"] = param("outWT", [128, 3], bf16)
    P["y3"] = nc.declare_dram_parameter("y3", [3, R], bf16, isOutput=True)
    if _DBG:
        for nm, shp, dt in [("d_h1", [128, R], bf16),
                            ("d_g0", [128, 512 * NN], f32),
                            ("d_s0", [128, 512 * NN], f32),
                            ("d_h0na", [128, B * NN], bf16),
                            ("d_pre0", [128, NN * 512], bf16)]:
            P[nm] = nc.declare_dram_parameter(nm, shp, dt, isOutput=True)

    with tile.TileContext(nc) as tc:
        _emit(nc, tc, mybir, P)
    nc.compile()
    return nc


def _emit(nc, tc, mybir, P):
    from contextlib import ExitStack

    f32 = mybir.dt.float32
    f32r = mybir.dt.float32r
    bf16 = mybir.dt.bfloat16
    AF = mybir.ActivationFunctionType
    Alu = mybir.AluOpType

    with ExitStack() as top:
        wpool = top.enter_context(tc.tile_pool(name="weights", bufs=1))
        persist = top.enter_context(tc.tile_pool(name="persist", bufs=1))
        h0pool = top.enter_context(tc.tile_pool(name="h0ring2", bufs=2))
        nfpool = top.enter_context(tc.tile_pool(name="nfring", bufs=2))
        scr = top.enter_context(tc.tile_pool(name="scr", bufs=2))
        nascr = top.enter_context(tc.tile_pool(name="nascr", bufs=3))
        h0ring = top.enter_context(tc.tile_pool(name="h0ring", bufs=3))
        cpool = top.enter_context(tc.tile_pool(name="cstate", bufs=2))
        im_pool = top.enter_context(tc.tile_pool(name="im", bufs=3))
        pta = top.enter_context(tc.tile_pool(name="pta", bufs=1, space="PSUM"))
        ppp = top.enter_context(tc.tile_pool(name="ppp", bufs=2, space="PSUM"))
        pna0 = top.enter_context(tc.tile_pool(name="pna0", bufs=2, space="PSUM"))
        pna1 = top.enter_context(tc.tile_pool(name="pna1", bufs=1, space="PSUM"))

        def wload(name, shape, dtype=f32):
            t = wpool.tile(list(shape), dtype, tag=name, name=name)
            nc.sync.dma_start(t[:], P[name][:])
            return t

        w0comb_t = wload("w0comb", [108, 768], bf16)
        lvic_t = wload("lvic", [75, 32], bf16)
        vicb_t = wload("vicb", [32, 1])
        xt = persist.tile([108, R], bf16, tag="xt")
        nc.sync.dma_start(xt[32:48, :], P["beat_bc"][:])
        nc.sync.dma_start(xt[48:96, :], P["e48"][:])
        nc.sync.dma_start(xt[96:108, :], P["chord_bc"][:])
        fp8 = mybir.dt.float8e4
        w1dr_t = wload("w1dr", [128, 2 * 768], fp8)
        w1dr3 = w1dr_t[:].rearrange("p (i m) -> p i m", i=2)
        b1_t = wload("b1t", [128, 6])
        lnfdr_t = wload("lnfdr", [128, 2 * 512], fp8)
        lnfdr3 = lnfdr_t[:].rearrange("p (i m) -> p i m", i=2)
        lhh0_t = wload("lhh0", [128, 512], bf16)
        lih1_t = wload("lih1", [128, 512], bf16)
        lhh1_t = wload("lhh1", [128, 512], bf16)
        ident_t = wload("ident", [128, 128], bf16)
        nb1bc_t = wload("nb1bc", [128, 512], bf16)
        outWT_t = wload("outWT", [128, 3], bf16)

        # persistent tiles
        h1All = persist.tile([128, R], bf16, tag="h1All")
        pre0 = persist.tile([128, NN * 512], bf16, tag="pre0")
        ysb = persist.tile([3, R], bf16, tag="ysb")

        pre0_r = pre0[:].rearrange("p (n q b) -> p n q b", q=4, b=B)

        # ---- TA block emitters -----------------------------------------
        h0cur = {}
        nfcur = {}

        def ta_conv(blk):
            sl = slice(blk * BLK, (blk + 1) * BLK)
            im_t = im_pool.tile([75, BLK], bf16, tag="imblk", name="imblk")
            nc.sync.dma_start(im_t[:], P["im2colT"][:, sl])
            vps = pta.tile([32, BLK], f32, tag="pg", name="vps")
            nc.tensor.matmul(vps[:], lvic_t[:], im_t[:])
            nc.scalar.activation(xt[0:32, sl], vps[:], AF.Tanh,
                                 bias=vicb_t[:, 0:1])

        def ta_l0_half(blk, half):
            sl = slice(blk * BLK, (blk + 1) * BLK)
            pio = pta.tile([128, 2 * BLK], f32, tag="pio", name="pio")
            pg = pta.tile([128, BLK], f32, tag="pg", name="pg")
            nc.tensor.matmul(pio[:, 0:BLK],
                             w0comb_t[:, half * 128:(half + 1) * 128],
                             xt[:, sl])
            nc.tensor.matmul(pio[:, BLK:2 * BLK],
                             w0comb_t[:, (4 + half) * 128:(5 + half) * 128],
                             xt[:, sl])
            nc.tensor.matmul(pg[:],
                             w0comb_t[:, (2 + half) * 128:(3 + half) * 128],
                             xt[:, sl])
            sio = scr.tile([128, 2 * BLK], f32, tag="sio")
            nc.scalar.activation(sio[:, 0:BLK], pio[:, 0:BLK], AF.Sigmoid)
            nc.scalar.activation(sio[:, BLK:2 * BLK], pio[:, BLK:2 * BLK],
                                 AF.Sigmoid)
            tg = scr.tile([128, BLK], f32, tag="tg")
            nc.scalar.activation(tg[:], pg[:], AF.Tanh)
            c2 = scr.tile([128, BLK], f32, tag="c2")
            nc.gpsimd.tensor_tensor(c2[:], sio[:, 0:BLK], tg[:], Alu.mult)
            tc2 = scr.tile([128, BLK], f32, tag="tc2")
            nc.scalar.activation(tc2[:], c2[:], AF.Tanh)
            if half == 0:
                h0cur[0] = h0pool.tile([128, 2 * BLK], fp8, tag="h0T",
                                       name="h0T")
            h0t = h0cur[0]
            nc.vector.tensor_tensor(h0t[:, half * BLK:(half + 1) * BLK],
                                    sio[:, BLK:2 * BLK], tc2[:], Alu.mult)

        def ta_l1_half(blk, half):
            pio = pta.tile([128, 2 * BLK], f32, tag="pio", name="bpio")
            pg = pta.tile([128, BLK], f32, tag="pg", name="bpg")
            h0r3 = h0cur[0][:].rearrange("p (i n) -> p i n", i=2)
            from concourse import mybir as _mb
            dr = _mb.MatmulPerfMode.DoubleRow
            for q, cols in ((half, slice(0, BLK)),
                            (4 + half, slice(BLK, 2 * BLK))):
                qs = slice(q * 128, (q + 1) * 128)
                nc.tensor.matmul(pio[:, cols], w1dr3[:, :, qs], h0r3,
                                 perf_mode=dr, start=True, stop=True)
            qs = slice((2 + half) * 128, (3 + half) * 128)
            nc.tensor.matmul(pg[:], w1dr3[:, :, qs], h0r3,
                             perf_mode=dr, start=True, stop=True)
            sio = scr.tile([128, 2 * BLK], f32, tag="bsio")
            nc.scalar.activation(sio[:, 0:BLK], pio[:, 0:BLK], AF.Sigmoid,
                                 bias=b1_t[:, half:half + 1])
            nc.scalar.activation(sio[:, BLK:2 * BLK], pio[:, BLK:2 * BLK],
                                 AF.Sigmoid, bias=b1_t[:, 4 + half:5 + half])
            tg = scr.tile([128, BLK], f32, tag="btg")
            nc.scalar.activation(tg[:], pg[:], AF.Tanh,
                                 bias=b1_t[:, 2 + half:3 + half])
            c2 = scr.tile([128, BLK], f32, tag="bc2")
            nc.gpsimd.tensor_tensor(c2[:], sio[:, 0:BLK], tg[:], Alu.mult)
            tc2 = scr.tile([128, BLK], f32, tag="btc2")
            nc.scalar.activation(tc2[:], c2[:], AF.Tanh)
            if half == 0:
                nfcur[0] = nfpool.tile([128, 2 * BLK], fp8, tag="nfT",
                                       name="nfT")
            nft = nfcur[0]
            nc.vector.tensor_tensor(nft[:, half * BLK:(half + 1) * BLK],
                                    sio[:, BLK:2 * BLK], tc2[:], Alu.mult)

        def pre0_block(blk):
            """pre0[:, blk notes] = lnf0@nf0 + lnf1@nf1 (psum) + presh."""
            psh = im_pool.tile([128, 4 * 512], bf16, tag="psh", name="psh")
            nc.sync.dma_start(psh[:], P["presh"][:, blk * 2048:(blk + 1) * 2048])
            psh_r = psh[:].rearrange("p (n q b) -> p n q b", q=4, b=B)
            nf3 = nfcur[0][:].rearrange("p (i n) -> p i n", i=2)
            from concourse import mybir as _mb
            dr = _mb.MatmulPerfMode.DoubleRow
            for q in range(4):
                qs = slice(q * 128, (q + 1) * 128)
                pp = ppp.tile([128, BLK], f32, tag="pp", name="pp")
                nc.tensor.matmul(pp[:], lnfdr3[:, :, qs], nf3,
                                 perf_mode=dr, start=True, stop=True)
                nsl = slice(4 * blk, 4 * blk + 4)
                pp_r = pp[:].rearrange("p (n b) -> p n b", b=B)
                nc.vector.tensor_tensor(pre0_r[:, nsl, q, :], pp_r,
                                        psh_r[:, :, q, :], Alu.add)

        # ---- NoteAxis --------------------------------------------------
        c_prev = [None, None]
        na_ps0 = {}
        h0_ring = {}

        def open_ps0(n):
            ps0 = pna0.tile([128, 512], f32, tag="na0", name="ps0")
            nc.tensor.matmul(ps0[:], ident_t[:],
                             pre0[:, n * 512:(n + 1) * 512],
                             start=True, stop=(n == 0))
            na_ps0[n] = ps0

        def na_step(n):
            ns = slice(n * B, (n + 1) * B)
            pns = slice((n - 1) * B, n * B)
            # ps1 openers: bias preload + h1 recurrence
            ps1 = pna1.tile([128, 512], f32, tag="na1", name="ps1")
            nc.tensor.matmul(ps1[:], ident_t[:], nb1bc_t[:],
                             start=True, stop=False)
            if n > 0:
                for q in range(4):
                    qs = slice(q * 128, (q + 1) * 128)
                    nc.tensor.matmul(ps1[:, qs], lhh1_t[:, qs],
                                     h1All[:, pns], start=False, stop=False)
            # ps0 close: h0 recurrence (latency-critical chain)
            hp = tc.high_priority(300)
            hp.__enter__()
            ps0 = na_ps0.pop(n)
            if n > 0:
                h0p = h0_ring.pop(n - 1)
                for q in range(4):
                    qs = slice(q * 128, (q + 1) * 128)
                    nc.tensor.matmul(ps0[:, qs], lhh0_t[:, qs],
                                     h0p[:], start=False, stop=(q == 3))
            h0r = h0ring.tile([128, B], bf16, tag="h0r", name="h0r")
            h0_ring[n] = h0r
            c_prev[0] = _lstm_nl(nc, nascr, cpool, mybir, ps0,
                                 c_prev[0], h0r[:], tag="L0")
            for q in range(4):
                qs = slice(q * 128, (q + 1) * 128)
                nc.tensor.matmul(ps1[:, qs], lih1_t[:, qs], h0r[:],
                                 start=False, stop=(q == 3))
            c_prev[1] = _lstm_nl(nc, nascr, cpool, mybir, ps1,
                                 c_prev[1], h1All[:, ns], tag="L1")
            hp.__exit__(None, None, None)
            if n + 1 < NN:
                open_ps0(n + 1)

        def out_block(blk):
            sl = slice(blk * BLK, (blk + 1) * BLK)
            pox = pta.tile([3, BLK], f32, tag="pg", name="pox")
            nc.tensor.matmul(pox[:], outWT_t[:], h1All[:, sl])
            nc.vector.tensor_copy(ysb[:, sl], pox[:])

        # ---- interleaved pipeline: TA(blk) runs 3 blocks ahead of NA ---
        for blk in range(NBLK + 3):
            chunks = [lambda b=blk: ta_l0_half(b, 0),
                      lambda b=blk: ta_l0_half(b, 1),
                      lambda b=blk: ta_l1_half(b, 0),
                      lambda b=blk: ta_l1_half(b, 1)]
            if blk < NBLK:
                ta_conv(blk)
            for j in range(4):
                if blk < NBLK:
                    chunks[j]()
                if blk >= 3:
                    na_step(4 * (blk - 3) + j)
            if blk < NBLK:
                pre0_block(blk)
            if blk == 0:
                open_ps0(0)
        for blk in range(NBLK):
            out_block(blk)
        nc.sync.dma_start(P["y3"][:], ysb[:])
        if _DBG:
            nc.sync.dma_start(P["d_h1"][:], h1All[:])
            nc.sync.dma_start(P["d_pre0"][:], pre0[:])


def _lstm_nl(nc, scr, cpool, mybir, ps, c_prev, h_out, tag, dbg=None):
    """Gate nonlinearity + state update for one NoteAxis layer-step.

    One sigmoid covers all four gate blocks (i, f, g, o); tanh(g) is
    recovered as 2*sigmoid(2g)-1 with g-gate rows pre-doubled on the host.
    sf*c_prev runs on GPSIMD off the critical path.  Returns the new c tile.
    """
    f32 = mybir.dt.float32
    AF = mybir.ActivationFunctionType
    Alu = mybir.AluOpType

    s = scr.tile([128, 512], f32, tag=f"{tag}s")
    nc.scalar.activation(s[:], ps[:], AF.Sigmoid)
    if dbg is not None:
        dparam, dn = dbg
        nc.sync.dma_start(dparam[:, 512 * dn:512 * (dn + 1)], s[:])
    si, sf, sg, so = (s[:, 128 * k:128 * (k + 1)] for k in range(4))
    gt = scr.tile([128, 128], f32, tag=f"{tag}gt")
    nc.vector.tensor_scalar(gt[:], sg, 2.0, -1.0, Alu.mult, Alu.add)

    c_new = cpool.tile([128, 128], f32, tag=f"{tag}c")
    if c_prev is None:
        nc.vector.tensor_tensor(c_new[:], si, gt[:], Alu.mult)
    else:
        t2 = scr.tile([128, 128], f32, tag=f"{tag}t2")
        nc.vector.tensor_tensor(t2[:], sf, c_prev[:], Alu.mult)
        t1 = scr.tile([128, 128], f32, tag=f"{tag}t1")
        nc.vector.tensor_tensor(t1[:], si, gt[:], Alu.mult)
        nc.vector.tensor_tensor(c_new[:], t1[:], t2[:], Alu.add)
    tcn = scr.tile([128, 128], f32, tag=f"{tag}tc")
    nc.scalar.activation(tcn[:], c_new[:], AF.Tanh)
    nc.vector.tensor_tensor(h_out, so, tcn[:], Alu.mult)
    return c_new


# --------------------------------------------------------------------------
# host side
# --------------------------------------------------------------------------

def _host_prep_weights(inp):
    import ml_dtypes

    f32 = np.float32
    bf16 = ml_dtypes.bfloat16

    W0 = np.asarray(inp["ta_Wih0"], f32)          # [1024, 73]
    sel = np.r_[0:256, 512:768, 768:1024]
    W0s = W0[sel]                                  # [768, 73] rows i,g,o
    b0s = (np.asarray(inp["ta_bih0"], f32) + np.asarray(inp["ta_bhh0"], f32))[sel]

    n = np.arange(NN)
    const_feat = np.zeros((13, NN), f32)
    const_feat[0] = n / NN
    const_feat[1 + (n % OCT), n] = 1.0

    beat_W = np.asarray(inp["beat_W"], f32)        # [16, 16]
    beat_b = np.asarray(inp["beat_b"], f32)
    gn = (W0s[:, 0:13] @ const_feat
          + (b0s + W0s[:, 13:29] @ beat_b)[:, None])        # [768, 48]
    Wbeat = W0s[:, 13:29] @ beat_W                 # [768, 16]
    Wvic = W0s[:, 29:61]                           # [768, 32]
    Wchord = W0s[:, 61:73]                         # [768, 12]
    w0comb = np.concatenate(
        [Wvic.T, Wbeat.T, gn.T, Wchord.T], axis=0
    ).astype(f32)                                  # [108, 768]

    vic_W = np.asarray(inp["vic_W"], f32)          # [32, 3, 25]
    lvic = vic_W.reshape(32, 75).T.copy()          # [75, 32] rows (c*25+s)
    vicb = np.asarray(inp["vic_b"], f32).reshape(32, 1)

    W1 = np.asarray(inp["ta_Wih1"], f32)[sel]      # [768, 256]
    b1s = (np.asarray(inp["ta_bih1"], f32) + np.asarray(inp["ta_bhh1"], f32))[sel]
    w1T = W1.T.astype(f32)                         # [256, 768]
    b1t = b1s.reshape(6, 128).T.copy()             # [128, 6]

    # sigma-trick: tanh(g) = 2*sigmoid(2g)-1, so double every g-gate row
    # (cols 256:384 of the transposed layouts) including the bias.
    def dbl_g(wT):
        wT = wT.copy()
        wT[:, 256:384] *= 2.0
        return wT

    naW0 = np.asarray(inp["na_Wih0"], f32)         # [512, 259]
    lnf = dbl_g(naW0[:, 0:256].T).astype(bf16)     # [256, 512]
    lhh0 = dbl_g(np.asarray(inp["na_Whh0"], f32).T).astype(bf16)
    lih1 = dbl_g(np.asarray(inp["na_Wih1"], f32).T).astype(bf16)
    lhh1 = dbl_g(np.asarray(inp["na_Whh1"], f32).T).astype(bf16)
    nb1 = (np.asarray(inp["na_bih1"], f32) + np.asarray(inp["na_bhh1"], f32))
    nb1d = dbl_g(nb1[None, :])[0]                  # [512]
    # nb1bc[p, q*128+b] = nb1d[q*128+p]
    nb1bc = np.broadcast_to(
        nb1d.reshape(4, 128).T[:, :, None], (128, 4, 128)
    ).reshape(128, 512).astype(bf16)

    outWT = np.asarray(inp["out_W"], f32).T.astype(bf16)     # [128, 3]

    return {
        "w0comb": w0comb.astype(bf16), "lvic": lvic.astype(bf16),
        "vicb": vicb,
        "w1dr": np.ascontiguousarray(
            w1T.reshape(2, 128, 768).transpose(1, 0, 2)
        ).reshape(128, 2 * 768).astype(ml_dtypes.float8_e4m3fn),
        "b1t": b1t,
        "lnfdr": np.ascontiguousarray(
            np.asarray(lnf, f32).reshape(2, 128, 512).transpose(1, 0, 2)
        ).reshape(128, 2 * 512).astype(ml_dtypes.float8_e4m3fn),
        "lhh0": lhh0, "lih1": lih1, "lhh1": lhh1,
        "ident": np.eye(128, dtype=f32).astype(bf16), "nb1bc": nb1bc,
        "outWT": outWT,
    }


def _host_prep_core(inp, note, beat, cond):
    """Per-core input gathering. note [B,48,3] etc."""
    import ml_dtypes

    f32 = np.float32
    bf16 = ml_dtypes.bfloat16
    pn = np.zeros((B, 72, 3), f32)
    pn[:, 12:60, :] = note
    # im2colT[(c*25+s), (n, b)] = pn[b, n+s, c]
    win = np.stack([pn[:, s:s + 48, :] for s in range(25)], axis=0)  # [25,B,48,3]
    im2colT = np.ascontiguousarray(win.transpose(3, 0, 2, 1)).reshape(75, R)

    beat_bc = np.ascontiguousarray(
        np.broadcast_to(beat.T[:, None, :], (16, NN, B))
    ).reshape(16, R)
    e48 = np.repeat(np.eye(48, dtype=f32), B, axis=1)        # [48, R]
    chord = (note[:, :, 0] / 4.0).reshape(B, OCT, 4).sum(axis=2)  # [B, 12]
    chord_bc = np.ascontiguousarray(
        np.broadcast_to(chord.T[:, None, :], (12, NN, B))
    ).reshape(12, R)

    sh = np.zeros((B, NN, 3), f32)
    sh[:, 1:, :] = cond[:, :-1, :]
    # presh[u,(n,b)] = Wsh @ shifted + bias0, then g-rows doubled,
    # laid out [p, (n, q, b)] with p = unit-within-chunk.
    naW0 = np.asarray(inp["na_Wih0"], f32)
    Wsh = naW0[:, 256:259]                          # [512, 3]
    nb0 = (np.asarray(inp["na_bih0"], f32) + np.asarray(inp["na_bhh0"], f32))
    G = np.einsum("uk,bnk->unb", Wsh, sh) + nb0[:, None, None]  # [512,n,b]
    G[256:384] *= 2.0
    presh = np.ascontiguousarray(
        G.reshape(4, 128, NN, B).transpose(1, 2, 0, 3)
    ).reshape(128, NN * 512).astype(bf16)

    return {
        "im2colT": im2colT.astype(bf16), "beat_bc": beat_bc.astype(bf16),
        "e48": e48.astype(bf16), "chord_bc": chord_bc.astype(bf16),
        "presh": presh,
    }


def _host_finish(y3, out_b):
    """y3 [3, R] raw out-projection -> [B, 48, 3] with bias + sigmoid."""
    v = y3.astype(np.float64).reshape(3, NN, B).transpose(2, 1, 0)
    v = v + out_b[None, None, :]
    v[:, :, 0:2] = 1.0 / (1.0 + np.exp(-v[:, :, 0:2]))
    return v.astype(np.float32)


def kernel(**inputs):
    from concourse.bass_utils import run_bass_kernel_spmd

    if "prog" not in _PROGRAM_CACHE:
        _PROGRAM_CACHE["prog"] = _build_program()
    nc = _PROGRAM_CACHE["prog"]

    wmap = _host_prep_weights(inputs)
    note = np.asarray(inputs["note_input"], np.float32)
    beat = np.asarray(inputs["beat_in"], np.float32)
    cond = np.asarray(inputs["condition_notes"], np.float32)
    out_b = np.asarray(inputs["out_b"], np.float64)

    in_maps = []
    for c in range(N_CORES):
        bs = slice(c * B, (c + 1) * B)
        m = dict(wmap)
        m.update(_host_prep_core(inputs, note[bs], beat[bs], cond[bs]))
        in_maps.append(m)

    res = run_bass_kernel_spmd(nc, in_maps, list(range(N_CORES)))
    outs = [_host_finish(res.results[c]["y3"], out_b) for c in range(N_CORES)]
    return np.concatenate(outs, axis=0).astype(np.float32)
